# revision 1
# baseline (speedup 1.0000x reference)
"""LIF spiking network forward (nn_LIFSG) on 8 Trainium2 NeuronCores.

Math (per reference):
    I = einsum('bti,oi->bto', spikes, W)         # GEMM
    u_t = decay * v_{t-1} + I_t                  # leaky integrate
    s_t = (u_t - 1 > 0)                          # spike
    v_t = u_t * (1 - s_t)                        # reset to zero

Sharding: data-parallel over B (32 batches -> 4 per core). Each core:
  - GEMM as I[o, t] = (W^T tile).T @ (spikes^T tile) on the PE array.
    W is split into 3 bf16 terms (hi/mid/lo); spikes are binary so every
    bf16 product is exact -> fp32-exact GEMM in PSUM. Weight tiles are
    reused across the 4 batches (8 PSUM banks held) to hide LDWEIGHTS.
  - The T=1000 recurrence runs as 1000 fused custom-DVE instructions
    (one LIF step per instruction over all 2048 lanes of the core):
        u_new = select(u_old < nextafter(1), u_old, 0) * decay + I_t
    This is the critical path (~145 ns/dependent step on HW), so time
    chunks are graduated (128/372/500) to start the chain early, and
    spike extraction is moved to the Scalar engine.
  - Spikes extracted on ACT: s = relu(sign(u - 1))  (exact {0,1}),
    streamed out as [b, o, t]; the host transposes back to (B, T, n_out)
    during unshard.

Host-side work is limited to sharding/layout prep (transpose + dtype
cast + W splitting) and the inverse gather; all FLOPs run on device.
"""

import sys

sys.path.insert(0, "/opt/trn_rl_repo")

import numpy as np
import ml_dtypes

import concourse.bacc as bacc
import concourse.tile as tile
import concourse.mybir as mybir
import concourse.dve_ops as dve_ops
from concourse.dve_ops import DveOp
from concourse.dve_spec import C0, C1, Spec, Src0, Src1, Zero, lower, select
from concourse.dve_uop import DveOpSpec
from concourse.bass_utils import run_bass_kernel_spmd

# ---------------- problem constants (hardcoded from spec) ----------------
B, T, N_IN, N_OUT = 32, 1000, 1024, 512
N_CORES = 8
B_SH = B // N_CORES          # 4 batches per core
DECAY = float(np.exp(-1.0 / 20.0))
# u < nextafter(1.0)  <=>  u <= 1.0 in fp32
THRESH_LT = float(np.nextafter(np.float32(1.0), np.float32(np.inf)))

CH_LIST = [128, 372, 500]    # graduated: small head starts the chain early (HW-measured best order)
CH_MAX = max(CH_LIST)
N_IT = N_IN // 128           # 8 contraction tiles
N_OT = N_OUT // 128          # 4 output-partition tiles
LANES = B_SH * N_OT          # 16 scan lanes per core (free dim)
N_SPLIT = 3                  # bf16 splits of W

AF = mybir.ActivationFunctionType


# ---------------- custom DVE op: one LIF step per instruction ----------------
def _lif_ref(in0, in1, c0, c1, c2):
    y = np.where(in0.astype(np.float32) < c1, in0, 0.0).astype(np.float32)
    return (y * np.float32(c0) + in1.astype(np.float32)).astype(np.float32)


_LIF_SPEC = Spec(body=select(Src0 < C1, Src0, Zero) * C0 + Src1, reference=_lif_ref)
_LIF_NAME = "LIF_STEP_ANT"


def _register_lif_op() -> DveOp:
    if _LIF_NAME in dve_ops._SUB_OPCODE_FOR_NAME:
        for op in dve_ops.OPS:
            if op.name == _LIF_NAME:
                return op
    opcode = dve_ops._CUSTOM_DVE_ROW_BASE + len(dve_ops.OPS)
    assert opcode < 0x20
    dve_ops._SUB_OPCODE_FOR_NAME[_LIF_NAME] = opcode
    shas = {}
    for ver in ("v3", "v4"):
        tmp = DveOpSpec(
            name=_LIF_NAME, opcode=opcode, uops=lower(_LIF_SPEC, ver=ver), rd1_en=True
        )
        shas[ver] = tmp.sha(ver)
    op = DveOp(_LIF_NAME, _LIF_SPEC, subdim=False, uops_sha=shas)
    dve_ops.OPS.append(op)
    dve_ops.CUSTOM_DVE_SPECS[_LIF_NAME] = _LIF_SPEC
    return op


# ---------------- device kernel ----------------
def _build_kernel():
    LIF = _register_lif_op()
    nc = bacc.Bacc("TRN2", target_bir_lowering=False, debug=False, num_devices=N_CORES)
    # Register a -1.0 const AP (used as the ACT Sign bias); mirrors the
    # built-in const registration in Bass.__init__.
    _cm1 = nc.alloc_sbuf_tensor("const-float32-neg1", [128, 1], mybir.dt.float32)
    nc.gpsimd.memset(_cm1.ap(), -1.0)
    nc.const_aps.aps[(mybir.dt.float32, -1.0)] = _cm1.ap()
    nc.all_engine_barrier()
    xT = nc.dram_tensor("xT", [B_SH, N_IN, T], mybir.dt.bfloat16, kind="ExternalInput")
    wts = nc.dram_tensor(
        "wts", [N_SPLIT, N_IN, N_OUT], mybir.dt.bfloat16, kind="ExternalInput"
    )
    out = nc.dram_tensor("out", [B_SH, N_OUT, T], mybir.dt.float32, kind="ExternalOutput")

    with tile.TileContext(nc) as tc:
        with (
            tc.tile_pool(name="wx", bufs=1) as wx_pool,
            tc.tile_pool(name="state", bufs=1) as state_pool,
            tc.tile_pool(name="spk", bufs=4) as spk_pool,
            tc.tile_pool(name="mm", bufs=8, space="PSUM") as psum_pool,
        ):
            # Stationary weights: [128p, split, it, o]. One DMA per split so
            # the s=0 matmuls only wait for the first 1MB.
            w_sb = wx_pool.tile([128, N_SPLIT, N_IT, N_OUT], mybir.dt.bfloat16, tag="w")
            wts_r = wts.rearrange("s (it p) o -> p s it o", p=128)
            for s in range(N_SPLIT):
                nc.sync.dma_start(w_sb[:, s], wts_r[:, s])
            # Spike inputs, all resident: one tile per batch [128p, it, T].
            # Split each DMA into head (chunk-0 columns) + rest so the first
            # chunk's matmuls start after ~1MB instead of 8MB of input DMA.
            head = CH_LIST[0]
            x_sb = []
            for b in range(B_SH):
                xt = wx_pool.tile(
                    [128, N_IT, T], mybir.dt.bfloat16, tag=f"x{b}", name=f"x{b}"
                )
                xTb = xT[b].rearrange("(it p) t -> p it t", p=128)
                nc.sync.dma_start(xt[:, :, :head], xTb[:, :, :head])
                nc.sync.dma_start(xt[:, :, head:], xTb[:, :, head:])
                x_sb.append(xt)

            # Scan state (u trajectory) and GEMM output, ping-pong per chunk
            # Per-parity max chunk length (ping-pong buffers sized to need)
            chmax = [max(c for i, c in enumerate(CH_LIST) if i % 2 == k) for k in range(2)]
            U = [
                state_pool.tile(
                    [128, LANES, chmax[k] + 1], mybir.dt.float32, tag=f"U{k}", name=f"U{k}"
                )
                for k in range(2)
            ]
            Ibuf = [
                state_pool.tile(
                    [128, LANES, chmax[k]], mybir.dt.float32, tag=f"I{k}", name=f"I{k}"
                )
                for k in range(2)
            ]
            zero_col = state_pool.tile([128, LANES], mybir.dt.float32, tag="z")
            nc.vector.memset(zero_col[:], 0.0)

            t0 = 0
            prev_ch = 0
            for ic, ch in enumerate(CH_LIST):
                pc = ic % 2
                # ---- GEMM for this chunk: I[o, t] per (b, ot) lane ----
                # Weight tile outer, batch inner: each loaded weight feeds 4
                # matmuls; 8 PSUM banks (2 ot x 4 b) accumulate concurrently.
                for half in range(2):
                    ots = (2 * half, 2 * half + 1)
                    pss = {
                        (ot, b): psum_pool.tile(
                            [128, ch], mybir.dt.float32, tag="ps", name="ps"
                        )
                        for ot in ots
                        for b in range(B_SH)
                    }
                    for s in range(N_SPLIT):
                        for it in range(N_IT):
                            for ot in ots:
                                w_ap = w_sb[:, s, it, ot * 128 : (ot + 1) * 128]
                                for b in range(B_SH):
                                    nc.tensor.matmul(
                                        pss[(ot, b)][:],
                                        w_ap,
                                        x_sb[b][:, it, t0 : t0 + ch],
                                        start=(s == 0 and it == 0),
                                        stop=(s == N_SPLIT - 1 and it == N_IT - 1),
                                    )
                    for ot in ots:
                        for b in range(B_SH):
                            lane = b * N_OT + ot
                            nc.scalar.copy(Ibuf[pc][:, lane, :ch], pss[(ot, b)][:])

                # ---- LIF chain: one custom-DVE instruction per timestep ----
                for j in range(ch):
                    if ic == 0 and j == 0:
                        prev = zero_col[:]
                    elif j == 0:
                        prev = U[1 - pc][:, :, prev_ch]
                    else:
                        prev = U[pc][:, :, j]
                    nc.vector._custom_dve(
                        LIF,
                        out=U[pc][:, :, j + 1],
                        in0=prev,
                        in1=Ibuf[pc][:, :, j],
                        s0=DECAY,
                        s1=THRESH_LT,
                    )

                # ---- spike extraction on ACT: s = relu(sign(u - 1)) ----
                # Last chunk: extract in two column-halves so the first half
                # (and its store) overlaps the still-running chain tail.
                pieces = (
                    [(1, ch // 2), (1 + ch // 2, ch - ch // 2)]
                    if ic == len(CH_LIST) - 1
                    else [(1, ch)]
                )
                for c0, clen in pieces:
                    for b in range(B_SH):
                        for ot in range(N_OT):
                            lane = b * N_OT + ot
                            st = spk_pool.tile(
                                [128, CH_MAX], mybir.dt.float32, tag="s", name="s"
                            )
                            nc.vector.tensor_scalar(
                                st[:, :clen],
                                U[pc][:, lane, c0 : c0 + clen],
                                1.0,
                                None,
                                mybir.AluOpType.is_gt,
                            )
                            nc.sync.dma_start(
                                out[
                                    b,
                                    ot * 128 : (ot + 1) * 128,
                                    t0 + c0 - 1 : t0 + c0 - 1 + clen,
                                ],
                                st[:, :clen],
                            )
                t0 += ch
                prev_ch = ch

    _dedupe_ldweights(nc)
    nc.compile()
    return nc


def _dedupe_ldweights(nc):
    """Remove back-to-back redundant Ldweights.

    The batch-inner GEMM loop issues 4 matmuls per weight tile; bass emits
    an Ldweights per matmul, so 3 of every 4 weight loads re-load the array
    with the bits it already holds (~75us of PE time). The PE keeps the
    stationary operand until the next Ldweights, so a duplicate load whose
    weights AP is identical to the previous one is a no-op -- drop it,
    provided it carries no semaphore waits/updates and only Matmult
    instructions sit in between (nothing else can clobber the array, and
    the weight tile in SBUF is written once at kernel start).
    """

    def _key(inst):
        a = inst.ins[0]
        try:
            return (a.memory_location().name, a.offset, str(a.ap))
        except Exception:
            return None

    removed = 0
    for blk in nc.m.functions[0].blocks:
        prev_key = None
        keep = []
        for inst in blk.instructions:
            if inst.opcode == "Ldweights":
                k = _key(inst)
                plain = not inst.sync_info and k is not None
                if plain and k == prev_key:
                    removed += 1
                    continue
                prev_key = k if plain else None
            elif inst.opcode != "Matmult":
                prev_key = None
            keep.append(inst)
        blk.instructions = keep
    return removed


_NC_CACHE = None


def _prep_inputs(input_spikes_seq: np.ndarray, W: np.ndarray):
    W32 = np.ascontiguousarray(np.asarray(W, dtype=np.float32).T)   # [n_in, n_out]
    w_hi = W32.astype(ml_dtypes.bfloat16)
    r1 = W32 - w_hi.astype(np.float32)
    w_mid = r1.astype(ml_dtypes.bfloat16)
    w_lo = (r1 - w_mid.astype(np.float32)).astype(ml_dtypes.bfloat16)
    wts = np.ascontiguousarray(np.stack([w_hi, w_mid, w_lo])[:N_SPLIT])

    x = np.asarray(input_spikes_seq, dtype=np.float32)
    in_maps = []
    for c in range(N_CORES):
        xs = x[c * B_SH : (c + 1) * B_SH]                           # [4, T, n_in]
        xs_T = np.ascontiguousarray(xs.transpose(0, 2, 1)).astype(ml_dtypes.bfloat16)
        in_maps.append({"xT": xs_T, "wts": wts})
    return in_maps


def kernel(input_spikes_seq: np.ndarray, W: np.ndarray) -> np.ndarray:
    global _NC_CACHE
    if _NC_CACHE is None:
        _NC_CACHE = _build_kernel()
    nc = _NC_CACHE

    in_maps = _prep_inputs(input_spikes_seq, W)
    res = run_bass_kernel_spmd(nc, in_maps, core_ids=list(range(N_CORES)))

    # ---- gather/unshard: [core][4, n_out, T] -> (B, T, n_out) ----
    outs = [r["out"] for r in res.results]
    full = np.concatenate(outs, axis=0)                             # [B, n_out, T]
    return np.ascontiguousarray(full.transpose(0, 2, 1))



# revision 2
# speedup vs baseline: 2.3055x; 2.3055x over previous
"""LIF spiking network forward (nn_LIFSG) on 8 Trainium2 NeuronCores.

Math (per reference):
    I = einsum('bti,oi->bto', spikes, W)         # GEMM
    u_t = decay * v_{t-1} + I_t                  # leaky integrate
    s_t = (u_t - 1 > 0)                          # spike
    v_t = u_t * (1 - s_t)                        # reset to zero

Sharding: data-parallel over B (32 batches -> 4 per core).

Design (measured ~106 us HW, vs 317 us for the previous version):
  - Weights are split into 2 bf16 terms (hi/mid).  Spikes are binary, so
    every bf16 product is exact; the only error is the ~2^-18-relative
    weight truncation.  CPU simulation of the full scan shows 2 mismatched
    spike elements of 16.4M (rel err 0.007, vs the 0.02 gate).  Dropping
    the third split cuts PE work by 33% (~107 us of matmul per core).
  - The T=1000 recurrence runs as 500 "LIF2" instructions on the Vector
    engine.  LIF2 is a hand-written 3-state DVE micro-op program (custom
    uop FSM seeded into dve_ops._COMPILE_CACHE) that computes TWO
    dependent LIF steps per instruction using all 8 ALU stages:
        state A: capture I_t into the block-3 swap flop (1 cycle)
        state B: u_t   = select(u_prev < th, u_prev, 0)*d + I_t   (swap)
                 u_t+1 = select(u_t    < th, u_t,    0)*d + I_t+1 (Src1)
    Per-instruction cost is fixed (~228 ns, SBUF round-trip bound), so
    halving the instruction count halves the serial chain: ~114 us.
  - The chain stores only odd-step u values; even-step u's are backfilled
    by one strided parallel DVE pass per chunk (off the dependent path,
    hidden in the chain's chunk-boundary waits).
  - Spike extraction (u > 1) runs on the otherwise idle GPSIMD engine;
    PSUM->SBUF copies (pair-interleaving I for LIF2's 2-cycle consume
    pattern) run on the Scalar engine; all off the critical path.
  - Time is cut into chunks (64,192,192,192,192,128,40): GEMM of chunk
    c+1 and x-DMA of chunk c+2 overlap the chain of chunk c.

Layouts per chunk (parity ping-pong pc):
  U[pc]   [128, chmax+1, 16]   u trajectory, t-major (contiguous lane rows)
  I2[pc]  [128, chmax/2, 32]   GEMM output, pair-interleaved (I_t,I_t+1)/lane
  St[pc]  [128, 16, chmax]     spikes, lane-major (contiguous DMA rows)
"""

import sys

sys.path.insert(0, "/opt/trn_rl_repo")

import numpy as np
import ml_dtypes

import concourse.bacc as bacc
import concourse.tile as tile
import concourse.mybir as mybir
import concourse.dve_ops as dve_ops
from concourse.dve_ops import DveOp
from concourse.dve_spec import C0, C1, Spec, Src0, Src1, Zero, lower, select
from concourse.dve_uop import (
    ENABLE,
    AluInp,
    AluOp,
    DelayInp,
    DveOpSpec,
    InpSel,
    OutPath,
    OutSel,
    Trigger,
    UopConfig,
)
from concourse.bass_utils import run_bass_kernel_spmd

# ---------------- problem constants (hardcoded from spec) ----------------
B, T, N_IN, N_OUT = 32, 1000, 1024, 512
N_CORES = 8
B_SH = B // N_CORES          # 4 batches per core
DECAY = float(np.exp(-1.0 / 20.0))
# u < nextafter(1.0)  <=>  u <= 1.0 in fp32
THRESH_LT = float(np.nextafter(np.float32(1.0), np.float32(np.inf)))

CH_LIST = [64, 192, 192, 192, 192, 128, 40]
assert sum(CH_LIST) == T and all(c % 2 == 0 for c in CH_LIST)
N_IT = N_IN // 128           # 8 contraction tiles
N_OT = N_OUT // 128          # 4 output-partition tiles
LANES = B_SH * N_OT          # 16 scan lanes per core
N_SPLIT = 2                  # bf16 splits of W


# ---------------- custom DVE op: one LIF step per instruction ----------------
def _lif_ref(in0, in1, c0, c1, c2):
    y = np.where(in0.astype(np.float32) < c1, in0, 0.0).astype(np.float32)
    return (y * np.float32(c0) + in1.astype(np.float32)).astype(np.float32)


_LIF_SPEC = Spec(body=select(Src0 < C1, Src0, Zero) * C0 + Src1, reference=_lif_ref)
_LIF_NAME = "LIF_STEP_ANT"


def _register_lif_op() -> DveOp:
    if _LIF_NAME in dve_ops._SUB_OPCODE_FOR_NAME:
        for op in dve_ops.OPS:
            if op.name == _LIF_NAME:
                return op
    opcode = dve_ops._CUSTOM_DVE_ROW_BASE + len(dve_ops.OPS)
    assert opcode < 0x20
    dve_ops._SUB_OPCODE_FOR_NAME[_LIF_NAME] = opcode
    from concourse.dve_uop import DveOpSpec as _DOS

    shas = {}
    for ver in ("v3", "v4"):
        tmp = _DOS(
            name=_LIF_NAME, opcode=opcode, uops=lower(_LIF_SPEC, ver=ver), rd1_en=True
        )
        shas[ver] = tmp.sha(ver)
    op = DveOp(_LIF_NAME, _LIF_SPEC, subdim=False, uops_sha=shas)
    dve_ops.OPS.append(op)
    dve_ops.CUSTOM_DVE_SPECS[_LIF_NAME] = _LIF_SPEC
    return op


# ------------- custom DVE op: two LIF steps per instruction (LIF2) -----------
_LIF2_NAME = "LIF2_STEP_ANT"


def _lif2_ref(in0, in1, c0, c1, c2):
    u = in0.astype(np.float32)
    out = np.zeros(in1.shape, np.float32)
    for k in range(2):
        u = np.where(u < np.float32(c1), u, np.float32(0.0)) * np.float32(c0) + in1[
            ..., k
        ].astype(np.float32)
        out[..., k] = u
    return out


def _lif2_mk_A(next_idx: int) -> UopConfig:
    """State A: consume one Src1 element (I_t), capture it in the block-3
    swap flop for the following B state; no output."""
    u = UopConfig()
    u.enable_input(InpSel.SRC_1, 2)  # delay chain 1 = I_t
    for b in range(3):
        u.datapath_config[b].pass_through_delay(1)
    blk = u.datapath_config[3]
    blk.enable_alu(AluOp.BYPASS, AluInp.PREV_DELAY_1, AluInp.PREV_DELAY_1)
    blk.swap_enable = ENABLE
    u.require_inp1 = ENABLE
    u.repeat_count = 1
    u.trigger = (Trigger.COUNT, Trigger.NONE, Trigger.NONE)
    u.next_uop = (next_idx, 0, 0)
    return u


def _lif2_mk_B() -> UopConfig:
    """State B: consume u_prev (Src0) + I_{t+1} (Src1); compute both LIF
    steps through the 8 ALU blocks; write u_{t+1}."""
    u = UopConfig()
    u.enable_input(InpSel.SRC_0, 1)    # chain 0 = u_prev
    u.enable_input(InpSel.SRC_1, 2)    # chain 1 = I_{t+1}
    u.enable_input(InpSel.CONST_0, 3)  # chain 2 = decay
    u.enable_input(InpSel.CONST_1, 4)  # chain 3 = thresh'
    u.enable_input(InpSel.ZERO, 5)     # chain 4 = 0.0
    dp = u.datapath_config
    dp[0].enable_alu(AluOp.IS_LT, AluInp.PREV_DELAY_0, AluInp.PREV_DELAY_3)
    dp[0].pass_through_delay(0, 1, 2, 3, 4)
    # SELECT: cond = PREV_ALU_OUT, src0 = else-value, src1 = then-value
    dp[1].enable_alu(AluOp.SELECT, AluInp.PREV_DELAY_4, AluInp.PREV_DELAY_0)
    dp[1].pass_through_delay(1, 2, 3, 4)
    dp[2].enable_alu(AluOp.MULTIPLY, AluInp.PREV_ALU_OUT, AluInp.PREV_DELAY_2)
    dp[2].pass_through_delay(1, 2, 3, 4)
    dp[3].enable_alu(AluOp.ADD, AluInp.PREV_ALU_OUT, AluInp.CURR_SWAP_OUT)
    dp[3].pass_through_delay(1, 2, 3, 4)
    dp[4].enable_alu(AluOp.IS_LT, AluInp.PREV_ALU_OUT, AluInp.PREV_DELAY_3)
    dp[4].pass_through_delay(1, 2, 4)
    dp[4].enable_delay_from_src(DelayInp.PREV_ALU_OUT, 5)   # u_t -> chain 5
    dp[5].enable_alu(AluOp.SELECT, AluInp.PREV_DELAY_4, AluInp.PREV_DELAY_5)
    dp[5].pass_through_delay(1, 2, 5)
    dp[6].enable_alu(AluOp.MULTIPLY, AluInp.PREV_ALU_OUT, AluInp.PREV_DELAY_2)
    dp[6].pass_through_delay(1, 5)
    dp[7].enable_alu(AluOp.ADD, AluInp.PREV_ALU_OUT, AluInp.PREV_DELAY_1)
    dp[7].pass_through_delay(5)
    u.enable_output(OutSel.ALU_OUT, OutPath.WR0_LO)         # u_{t+1}
    u.require_inp0 = ENABLE
    u.require_inp1 = ENABLE
    u.repeat_count = 1
    u.trigger = (Trigger.SRC_TENSOR_DONE, Trigger.COUNT, Trigger.NONE)
    u.next_uop = (0, 2, 0)
    return u


def register_lif2() -> DveOp:
    if _LIF2_NAME in dve_ops._SUB_OPCODE_FOR_NAME:
        for op in dve_ops.OPS:
            if op.name == _LIF2_NAME:
                return op
    opcode = dve_ops._CUSTOM_DVE_ROW_BASE + len(dve_ops.OPS)
    assert opcode < 0x20
    dve_ops._SUB_OPCODE_FOR_NAME[_LIF2_NAME] = opcode

    uops = [_lif2_mk_A(1), _lif2_mk_B(), _lif2_mk_A(1)]
    spec_hw = DveOpSpec(name=_LIF2_NAME, opcode=opcode, uops=uops, rd1_en=True)
    spec_hw.validate("v3")
    # Placeholder Spec: right leaves (Src0/Src1/C0/C1, no C2/C3) for the
    # call-site checks; reference carries the true 2-step semantics.
    ph = Spec(
        body=select(Src0 < C1, Src0, Zero) * C0 + Src1,
        reference=_lif2_ref,
    )
    op = DveOp(_LIF2_NAME, ph, subdim=False, uops_sha={"v3": spec_hw.sha("v3")})
    dve_ops.OPS.append(op)
    dve_ops.CUSTOM_DVE_SPECS[_LIF2_NAME] = ph
    dve_ops._COMPILE_CACHE[(_LIF2_NAME, "v3")] = spec_hw
    return op


# ---------------- device kernel ----------------
def _build_kernel(n_iter: int = 0):
    """n_iter=0: production kernel. n_iter>0: For_i-wrapped timing probe."""
    LIF = _register_lif_op()
    LIF2 = register_lif2()
    nc = bacc.Bacc("TRN2", target_bir_lowering=False, debug=False, num_devices=N_CORES)
    xT = nc.dram_tensor("xT", [B_SH, N_IN, T], mybir.dt.bfloat16, kind="ExternalInput")
    wts = nc.dram_tensor(
        "wts", [N_SPLIT, N_IN, N_OUT], mybir.dt.bfloat16, kind="ExternalInput"
    )
    out = nc.dram_tensor("out", [B_SH, N_OUT, T], mybir.dt.float32, kind="ExternalOutput")

    chmax = [0, 0]
    for i, c in enumerate(CH_LIST):
        chmax[i % 2] = max(chmax[i % 2], c)

    with tile.TileContext(nc) as tc:
        with (
            tc.tile_pool(name="wx", bufs=1) as wx_pool,
            tc.tile_pool(name="state", bufs=1) as state_pool,
            tc.tile_pool(name="mm", bufs=8, space="PSUM") as psum_pool,
        ):
            w_sb = wx_pool.tile([128, N_SPLIT, N_IT, N_OUT], mybir.dt.bfloat16, tag="w")
            xb4 = [
                wx_pool.tile(
                    [128, B_SH * N_IT, max(chmax)],
                    mybir.dt.bfloat16,
                    tag=f"x4{k}",
                    name=f"x4{k}",
                )
                for k in range(2)
            ]
            U = [
                state_pool.tile(
                    [128, chmax[k] + 1, LANES], mybir.dt.float32, tag=f"U{k}", name=f"U{k}"
                )
                for k in range(2)
            ]
            I2 = [
                state_pool.tile(
                    [128, chmax[k] // 2, 2 * LANES],
                    mybir.dt.float32,
                    tag=f"I{k}",
                    name=f"I{k}",
                )
                for k in range(2)
            ]
            St = [
                state_pool.tile(
                    [128, LANES, chmax[k]], mybir.dt.float32, tag=f"S{k}", name=f"S{k}"
                )
                for k in range(2)
            ]
            zero_col = state_pool.tile([128, LANES], mybir.dt.float32, tag="z")
            nc.vector.memset(zero_col[:], 0.0)

            def body():
                wts_r = wts.rearrange("s (it p) o -> p s it o", p=128)
                for s in range(N_SPLIT):
                    nc.sync.dma_start(w_sb[:, s], wts_r[:, s])
                xT_r = xT.rearrange("b (it p) t -> p (b it) t", p=128)

                def dma_x(k, t0, ch):
                    nc.sync.dma_start(xb4[k][:, :, :ch], xT_r[:, :, t0 : t0 + ch])

                out_r = [
                    out[b].rearrange("(ot p) t -> p ot t", p=128) for b in range(B_SH)
                ]
                t0s = np.cumsum([0] + CH_LIST).tolist()
                dma_x(0, t0s[0], CH_LIST[0])
                if len(CH_LIST) > 1:
                    dma_x(1, t0s[1], CH_LIST[1])

                prev_ch = 0
                for ic, ch in enumerate(CH_LIST):
                    pc = ic % 2
                    t0 = t0s[ic]
                    # ---- GEMM: weight tile outer, batch inner; 8 PSUM banks
                    # per half accumulate 16 matmuls (2 splits x 8 it) ----
                    for half in range(2):
                        ots = (2 * half, 2 * half + 1)
                        pss = {
                            (ot, b): psum_pool.tile(
                                [128, ch], mybir.dt.float32, tag="ps", name="ps"
                            )
                            for ot in ots
                            for b in range(B_SH)
                        }
                        for s in range(N_SPLIT):
                            for it in range(N_IT):
                                for ot in ots:
                                    w_ap = w_sb[:, s, it, ot * 128 : (ot + 1) * 128]
                                    for b in range(B_SH):
                                        nc.tensor.matmul(
                                            pss[(ot, b)][:],
                                            w_ap,
                                            xb4[pc][:, b * N_IT + it, :ch],
                                            start=(s == 0 and it == 0),
                                            stop=(s == N_SPLIT - 1 and it == N_IT - 1),
                                        )
                        # PSUM -> I2 pair-interleaved copies on ACT
                        for ot in ots:
                            for b in range(B_SH):
                                lane = b * N_OT + ot
                                nc.scalar.copy(
                                    I2[pc][:, : ch // 2, 2 * lane : 2 * lane + 2],
                                    pss[(ot, b)][:].rearrange("p (j k) -> p j k", k=2),
                                )
                    # prefetch x for chunk ic+2 (reuses this chunk's buffer)
                    if ic + 2 < len(CH_LIST):
                        dma_x(pc, t0s[ic + 2], CH_LIST[ic + 2])

                    # ---- dependent chain: ch/2 LIF2 instructions ----
                    for j in range(ch // 2):
                        t = 2 * j
                        if ic == 0 and j == 0:
                            prev = zero_col[:]
                        elif j == 0:
                            prev = U[1 - pc][:, prev_ch, :]
                        else:
                            prev = U[pc][:, t, :]
                        nc.vector._custom_dve(
                            LIF2,
                            out=U[pc][:, t + 2, :],
                            in0=prev,
                            in1=I2[pc][:, j, :],
                            s0=DECAY,
                            s1=THRESH_LT,
                        )

                    # ---- backfill even-step u's (odd rows); off the chain ----
                    if ic == 0:
                        nc.vector.memset(U[pc][:, 0, :], 0.0)
                    else:
                        nc.scalar.copy(U[pc][:, 0, :], U[1 - pc][:, prev_ch, :])
                    nc.vector._custom_dve(
                        LIF,
                        out=U[pc][:, 1:ch:2, :],
                        in0=U[pc][:, 0 : ch - 1 : 2, :],
                        in1=I2[pc][:, : ch // 2, 0::2],
                        s0=DECAY,
                        s1=THRESH_LT,
                    )

                    # ---- spike extraction on GPSIMD + output DMA ----
                    pieces = (
                        [(0, ch // 2), (ch // 2, ch - ch // 2)]
                        if ic == len(CH_LIST) - 1 and ch >= 8
                        else [(0, ch)]
                    )
                    for c0, clen in pieces:
                        for lane in range(LANES):
                            nc.gpsimd.tensor_scalar(
                                St[pc][:, lane, c0 : c0 + clen],
                                U[pc][:, c0 + 1 : c0 + 1 + clen, lane],
                                1.0,
                                None,
                                mybir.AluOpType.is_gt,
                            )
                        for b in range(B_SH):
                            nc.sync.dma_start(
                                out_r[b][:, :, t0 + c0 : t0 + c0 + clen],
                                St[pc][:, b * N_OT : (b + 1) * N_OT, c0 : c0 + clen],
                            )
                    prev_ch = ch

            if n_iter:
                with tc.For_i(0, n_iter) as _:
                    body()
            else:
                body()

    _dedupe_ldweights(nc)
    nc.compile()
    return nc


def _dedupe_ldweights(nc):
    """Remove back-to-back redundant Ldweights (the batch-inner GEMM loop
    issues 4 matmuls per weight tile; 3 of 4 emitted weight loads re-load
    identical bits). Safe when no sync info is attached and only Matmult
    instructions sit between."""

    def _key(inst):
        a = inst.ins[0]
        try:
            return (a.memory_location().name, a.offset, str(a.ap))
        except Exception:
            return None

    removed = 0
    for blk in nc.m.functions[0].blocks:
        prev_key = None
        keep = []
        for inst in blk.instructions:
            if inst.opcode == "Ldweights":
                k = _key(inst)
                plain = not inst.sync_info and k is not None
                if plain and k == prev_key:
                    removed += 1
                    continue
                prev_key = k if plain else None
            elif inst.opcode != "Matmult":
                prev_key = None
            keep.append(inst)
        blk.instructions = keep
    return removed


_NC_CACHE = None


def _prep_inputs(input_spikes_seq: np.ndarray, W: np.ndarray):
    W32 = np.ascontiguousarray(np.asarray(W, dtype=np.float32).T)   # [n_in, n_out]
    w_hi = W32.astype(ml_dtypes.bfloat16)
    r1 = W32 - w_hi.astype(np.float32)
    w_mid = r1.astype(ml_dtypes.bfloat16)
    wts = np.ascontiguousarray(np.stack([w_hi, w_mid]))

    x = np.asarray(input_spikes_seq, dtype=np.float32)
    in_maps = []
    for c in range(N_CORES):
        xs = x[c * B_SH : (c + 1) * B_SH]                           # [4, T, n_in]
        xs_T = np.ascontiguousarray(xs.transpose(0, 2, 1)).astype(ml_dtypes.bfloat16)
        in_maps.append({"xT": xs_T, "wts": wts})
    return in_maps


def kernel(input_spikes_seq: np.ndarray, W: np.ndarray) -> np.ndarray:
    global _NC_CACHE
    if _NC_CACHE is None:
        _NC_CACHE = _build_kernel()
    nc = _NC_CACHE

    in_maps = _prep_inputs(input_spikes_seq, W)
    res = run_bass_kernel_spmd(nc, in_maps, core_ids=list(range(N_CORES)))

    # ---- gather/unshard: [core][4, n_out, T] -> (B, T, n_out) ----
    outs = [r["out"] for r in res.results]
    full = np.concatenate(outs, axis=0)                             # [B, n_out, T]
    return np.ascontiguousarray(full.transpose(0, 2, 1))


# revision 4
# speedup vs baseline: 2.5565x; 1.1089x over previous
"""LIF spiking network forward (nn_LIFSG) on 8 Trainium2 NeuronCores.

Math (per reference):
    I = einsum('bti,oi->bto', spikes, W)         # GEMM
    u_t = decay * v_{t-1} + I_t                  # leaky integrate
    s_t = (u_t - 1 > 0)                          # spike
    v_t = u_t * (1 - s_t)                        # reset to zero

Sharding: data-parallel over B (32 batches -> 4 per core).

Design (measured ~106 us HW, vs 317 us for the previous version):
  - Weights are split into 2 bf16 terms (hi/mid).  Spikes are binary, so
    every bf16 product is exact; the only error is the ~2^-18-relative
    weight truncation.  CPU simulation of the full scan shows 2 mismatched
    spike elements of 16.4M (rel err 0.007, vs the 0.02 gate).  Dropping
    the third split cuts PE work by 33% (~107 us of matmul per core).
  - The T=1000 recurrence runs as 500 "LIF2" instructions on the Vector
    engine.  LIF2 is a hand-written 3-state DVE micro-op program (custom
    uop FSM seeded into dve_ops._COMPILE_CACHE) that computes TWO
    dependent LIF steps per instruction using all 8 ALU stages:
        state A: capture I_t into the block-3 swap flop (1 cycle)
        state B: u_t   = select(u_prev < th, u_prev, 0)*d + I_t   (swap)
                 u_t+1 = select(u_t    < th, u_t,    0)*d + I_t+1 (Src1)
    Per-instruction cost is fixed (~228 ns, SBUF round-trip bound), so
    halving the instruction count halves the serial chain: ~114 us.
  - The chain stores only odd-step u values; even-step u's are backfilled
    by one strided parallel DVE pass per chunk (off the dependent path,
    hidden in the chain's chunk-boundary waits).
  - Spike extraction (u > 1) runs on the otherwise idle GPSIMD engine;
    PSUM->SBUF copies (pair-interleaving I for LIF2's 2-cycle consume
    pattern) run on the Scalar engine; all off the critical path.
  - Time is cut into chunks (48,80,96,112,128,144,160,160,72; sized so the chain never waits long on any one GEMM chunk): GEMM of chunk
    c+1 and x-DMA of chunk c+2 overlap the chain of chunk c.

Layouts per chunk (parity ping-pong pc):
  U[pc]   [128, chmax+1, 16]   u trajectory, t-major (contiguous lane rows)
  I2[pc]  [128, chmax/2, 32]   GEMM output, pair-interleaved (I_t,I_t+1)/lane
  St[pc]  [128, 16, chmax]     spikes, lane-major (contiguous DMA rows)
"""

import sys

sys.path.insert(0, "/opt/trn_rl_repo")

import numpy as np
import ml_dtypes

import concourse.bacc as bacc
import concourse.tile as tile
import concourse.mybir as mybir
import concourse.dve_ops as dve_ops
from concourse.dve_ops import DveOp
from concourse.dve_spec import C0, C1, Spec, Src0, Src1, Zero, lower, select
from concourse.dve_uop import (
    ENABLE,
    AluInp,
    AluOp,
    DelayInp,
    DveOpSpec,
    InpSel,
    OutPath,
    OutSel,
    Trigger,
    UopConfig,
)
from concourse.bass_utils import run_bass_kernel_spmd

# ---------------- problem constants (hardcoded from spec) ----------------
B, T, N_IN, N_OUT = 32, 1000, 1024, 512
N_CORES = 8
B_SH = B // N_CORES          # 4 batches per core
DECAY = float(np.exp(-1.0 / 20.0))
# u < nextafter(1.0)  <=>  u <= 1.0 in fp32
THRESH_LT = float(np.nextafter(np.float32(1.0), np.float32(np.inf)))

CH_LIST = [48, 80, 96, 112, 128, 144, 160, 160, 72]
assert sum(CH_LIST) == T and all(c % 2 == 0 for c in CH_LIST)
N_IT = N_IN // 128           # 8 contraction tiles
N_OT = N_OUT // 128          # 4 output-partition tiles
LANES = B_SH * N_OT          # 16 scan lanes per core
N_SPLIT = 2                  # bf16 splits of W


# ---------------- custom DVE op: one LIF step per instruction ----------------
def _lif_ref(in0, in1, c0, c1, c2):
    y = np.where(in0.astype(np.float32) < c1, in0, 0.0).astype(np.float32)
    return (y * np.float32(c0) + in1.astype(np.float32)).astype(np.float32)


_LIF_SPEC = Spec(body=select(Src0 < C1, Src0, Zero) * C0 + Src1, reference=_lif_ref)
_LIF_NAME = "LIF_STEP_ANT"


def _register_lif_op() -> DveOp:
    if _LIF_NAME in dve_ops._SUB_OPCODE_FOR_NAME:
        for op in dve_ops.OPS:
            if op.name == _LIF_NAME:
                return op
    opcode = dve_ops._CUSTOM_DVE_ROW_BASE + len(dve_ops.OPS)
    assert opcode < 0x20
    dve_ops._SUB_OPCODE_FOR_NAME[_LIF_NAME] = opcode
    from concourse.dve_uop import DveOpSpec as _DOS

    shas = {}
    for ver in ("v3", "v4"):
        tmp = _DOS(
            name=_LIF_NAME, opcode=opcode, uops=lower(_LIF_SPEC, ver=ver), rd1_en=True
        )
        shas[ver] = tmp.sha(ver)
    op = DveOp(_LIF_NAME, _LIF_SPEC, subdim=False, uops_sha=shas)
    dve_ops.OPS.append(op)
    dve_ops.CUSTOM_DVE_SPECS[_LIF_NAME] = _LIF_SPEC
    return op


# ------------- custom DVE op: two LIF steps per instruction (LIF2) -----------
_LIF2_NAME = "LIF2_STEP_ANT"


def _lif2_ref(in0, in1, c0, c1, c2):
    u = in0.astype(np.float32)
    out = np.zeros(in1.shape, np.float32)
    for k in range(2):
        u = np.where(u < np.float32(c1), u, np.float32(0.0)) * np.float32(c0) + in1[
            ..., k
        ].astype(np.float32)
        out[..., k] = u
    return out


def _lif2_mk_A(next_idx: int) -> UopConfig:
    """State A: consume one Src1 element (I_t), capture it in the block-3
    swap flop for the following B state; no output."""
    u = UopConfig()
    u.enable_input(InpSel.SRC_1, 2)  # delay chain 1 = I_t
    for b in range(3):
        u.datapath_config[b].pass_through_delay(1)
    blk = u.datapath_config[3]
    blk.enable_alu(AluOp.BYPASS, AluInp.PREV_DELAY_1, AluInp.PREV_DELAY_1)
    blk.swap_enable = ENABLE
    u.require_inp1 = ENABLE
    u.repeat_count = 1
    u.trigger = (Trigger.COUNT, Trigger.NONE, Trigger.NONE)
    u.next_uop = (next_idx, 0, 0)
    return u


def _lif2_mk_B() -> UopConfig:
    """State B: consume u_prev (Src0) + I_{t+1} (Src1); compute both LIF
    steps through the 8 ALU blocks; write u_{t+1}."""
    u = UopConfig()
    u.enable_input(InpSel.SRC_0, 1)    # chain 0 = u_prev
    u.enable_input(InpSel.SRC_1, 2)    # chain 1 = I_{t+1}
    u.enable_input(InpSel.CONST_0, 3)  # chain 2 = decay
    u.enable_input(InpSel.CONST_1, 4)  # chain 3 = thresh'
    u.enable_input(InpSel.ZERO, 5)     # chain 4 = 0.0
    dp = u.datapath_config
    dp[0].enable_alu(AluOp.IS_LT, AluInp.PREV_DELAY_0, AluInp.PREV_DELAY_3)
    dp[0].pass_through_delay(0, 1, 2, 3, 4)
    # SELECT: cond = PREV_ALU_OUT, src0 = else-value, src1 = then-value
    dp[1].enable_alu(AluOp.SELECT, AluInp.PREV_DELAY_4, AluInp.PREV_DELAY_0)
    dp[1].pass_through_delay(1, 2, 3, 4)
    dp[2].enable_alu(AluOp.MULTIPLY, AluInp.PREV_ALU_OUT, AluInp.PREV_DELAY_2)
    dp[2].pass_through_delay(1, 2, 3, 4)
    dp[3].enable_alu(AluOp.ADD, AluInp.PREV_ALU_OUT, AluInp.CURR_SWAP_OUT)
    dp[3].pass_through_delay(1, 2, 3, 4)
    dp[4].enable_alu(AluOp.IS_LT, AluInp.PREV_ALU_OUT, AluInp.PREV_DELAY_3)
    dp[4].pass_through_delay(1, 2, 4)
    dp[4].enable_delay_from_src(DelayInp.PREV_ALU_OUT, 5)   # u_t -> chain 5
    dp[5].enable_alu(AluOp.SELECT, AluInp.PREV_DELAY_4, AluInp.PREV_DELAY_5)
    dp[5].pass_through_delay(1, 2, 5)
    dp[6].enable_alu(AluOp.MULTIPLY, AluInp.PREV_ALU_OUT, AluInp.PREV_DELAY_2)
    dp[6].pass_through_delay(1, 5)
    dp[7].enable_alu(AluOp.ADD, AluInp.PREV_ALU_OUT, AluInp.PREV_DELAY_1)
    dp[7].pass_through_delay(5)
    u.enable_output(OutSel.ALU_OUT, OutPath.WR0_LO)         # u_{t+1}
    u.require_inp0 = ENABLE
    u.require_inp1 = ENABLE
    u.repeat_count = 1
    u.trigger = (Trigger.SRC_TENSOR_DONE, Trigger.COUNT, Trigger.NONE)
    u.next_uop = (0, 2, 0)
    return u


def register_lif2() -> DveOp:
    if _LIF2_NAME in dve_ops._SUB_OPCODE_FOR_NAME:
        for op in dve_ops.OPS:
            if op.name == _LIF2_NAME:
                return op
    opcode = dve_ops._CUSTOM_DVE_ROW_BASE + len(dve_ops.OPS)
    assert opcode < 0x20
    dve_ops._SUB_OPCODE_FOR_NAME[_LIF2_NAME] = opcode

    uops = [_lif2_mk_A(1), _lif2_mk_B(), _lif2_mk_A(1)]
    spec_hw = DveOpSpec(name=_LIF2_NAME, opcode=opcode, uops=uops, rd1_en=True)
    spec_hw.validate("v3")
    # Placeholder Spec: right leaves (Src0/Src1/C0/C1, no C2/C3) for the
    # call-site checks; reference carries the true 2-step semantics.
    ph = Spec(
        body=select(Src0 < C1, Src0, Zero) * C0 + Src1,
        reference=_lif2_ref,
    )
    op = DveOp(_LIF2_NAME, ph, subdim=False, uops_sha={"v3": spec_hw.sha("v3")})
    dve_ops.OPS.append(op)
    dve_ops.CUSTOM_DVE_SPECS[_LIF2_NAME] = ph
    dve_ops._COMPILE_CACHE[(_LIF2_NAME, "v3")] = spec_hw
    return op


# ---------------- device kernel ----------------
def _build_kernel(n_iter: int = 0, ch_list=None):
    """n_iter=0: production kernel. n_iter>0: For_i-wrapped timing probe."""
    CH_LIST = ch_list or globals()["CH_LIST"]
    assert sum(CH_LIST) == T and all(c % 2 == 0 for c in CH_LIST)
    LIF = _register_lif_op()
    LIF2 = register_lif2()
    nc = bacc.Bacc("TRN2", target_bir_lowering=False, debug=False, num_devices=N_CORES)
    xT = nc.dram_tensor("xT", [B_SH, N_IN, T], mybir.dt.bfloat16, kind="ExternalInput")
    wts = nc.dram_tensor(
        "wts", [N_SPLIT, N_IN, N_OUT], mybir.dt.bfloat16, kind="ExternalInput"
    )
    out = nc.dram_tensor("out", [B_SH, N_OUT, T], mybir.dt.float32, kind="ExternalOutput")

    chmax = [0, 0]
    for i, c in enumerate(CH_LIST):
        chmax[i % 2] = max(chmax[i % 2], c)

    with tile.TileContext(nc) as tc:
        with (
            tc.tile_pool(name="wx", bufs=1) as wx_pool,
            tc.tile_pool(name="state", bufs=1) as state_pool,
            tc.tile_pool(name="mm", bufs=8, space="PSUM") as psum_pool,
        ):
            w_sb = wx_pool.tile([128, N_SPLIT, N_IT, N_OUT], mybir.dt.bfloat16, tag="w")
            xb4 = [
                wx_pool.tile(
                    [128, B_SH * N_IT, max(chmax)],
                    mybir.dt.bfloat16,
                    tag=f"x4{k}",
                    name=f"x4{k}",
                )
                for k in range(2)
            ]
            U = [
                state_pool.tile(
                    [128, chmax[k] + 1, LANES], mybir.dt.float32, tag=f"U{k}", name=f"U{k}"
                )
                for k in range(2)
            ]
            I2 = [
                state_pool.tile(
                    [128, chmax[k] // 2, 2 * LANES],
                    mybir.dt.float32,
                    tag=f"I{k}",
                    name=f"I{k}",
                )
                for k in range(2)
            ]
            St = [
                state_pool.tile(
                    [128, LANES, chmax[k]], mybir.dt.float32, tag=f"S{k}", name=f"S{k}"
                )
                for k in range(2)
            ]
            zero_col = state_pool.tile([128, LANES], mybir.dt.float32, tag="z")
            nc.vector.memset(zero_col[:], 0.0)

            def body():
                wts_r = wts.rearrange("s (it p) o -> p s it o", p=128)
                for s in range(N_SPLIT):
                    nc.sync.dma_start(w_sb[:, s], wts_r[:, s])
                xT_r = xT.rearrange("b (it p) t -> p (b it) t", p=128)

                def dma_x(k, t0, ch):
                    nc.sync.dma_start(xb4[k][:, :, :ch], xT_r[:, :, t0 : t0 + ch])

                out_r = [
                    out[b].rearrange("(ot p) t -> p ot t", p=128) for b in range(B_SH)
                ]
                t0s = np.cumsum([0] + CH_LIST).tolist()
                dma_x(0, t0s[0], CH_LIST[0])
                if len(CH_LIST) > 1:
                    dma_x(1, t0s[1], CH_LIST[1])

                prev_ch = 0
                for ic, ch in enumerate(CH_LIST):
                    pc = ic % 2
                    t0 = t0s[ic]
                    # ---- GEMM: weight tile outer, batch inner; 8 PSUM banks
                    # per half accumulate 16 matmuls (2 splits x 8 it) ----
                    for half in range(2):
                        ots = (2 * half, 2 * half + 1)
                        pss = {
                            (ot, b): psum_pool.tile(
                                [128, ch], mybir.dt.float32, tag="ps", name="ps"
                            )
                            for ot in ots
                            for b in range(B_SH)
                        }
                        for s in range(N_SPLIT):
                            for it in range(N_IT):
                                for ot in ots:
                                    w_ap = w_sb[:, s, it, ot * 128 : (ot + 1) * 128]
                                    for b in range(B_SH):
                                        nc.tensor.matmul(
                                            pss[(ot, b)][:],
                                            w_ap,
                                            xb4[pc][:, b * N_IT + it, :ch],
                                            start=(s == 0 and it == 0),
                                            stop=(s == N_SPLIT - 1 and it == N_IT - 1),
                                        )
                        # PSUM -> I2 pair-interleaved copies on ACT
                        for ot in ots:
                            for b in range(B_SH):
                                lane = b * N_OT + ot
                                nc.scalar.copy(
                                    I2[pc][:, : ch // 2, 2 * lane : 2 * lane + 2],
                                    pss[(ot, b)][:].rearrange("p (j k) -> p j k", k=2),
                                )
                    # prefetch x for chunk ic+2 (reuses this chunk's buffer)
                    if ic + 2 < len(CH_LIST):
                        dma_x(pc, t0s[ic + 2], CH_LIST[ic + 2])

                    # ---- dependent chain: ch/2 LIF2 instructions ----
                    for j in range(ch // 2):
                        t = 2 * j
                        if ic == 0 and j == 0:
                            prev = zero_col[:]
                        elif j == 0:
                            prev = U[1 - pc][:, prev_ch, :]
                        else:
                            prev = U[pc][:, t, :]
                        nc.vector._custom_dve(
                            LIF2,
                            out=U[pc][:, t + 2, :],
                            in0=prev,
                            in1=I2[pc][:, j, :],
                            s0=DECAY,
                            s1=THRESH_LT,
                        )

                    # ---- backfill even-step u's (odd rows); off the chain ----
                    if ic == 0:
                        nc.vector.memset(U[pc][:, 0, :], 0.0)
                    else:
                        nc.scalar.copy(U[pc][:, 0, :], U[1 - pc][:, prev_ch, :])
                    nc.vector._custom_dve(
                        LIF,
                        out=U[pc][:, 1:ch:2, :],
                        in0=U[pc][:, 0 : ch - 1 : 2, :],
                        in1=I2[pc][:, : ch // 2, 0::2],
                        s0=DECAY,
                        s1=THRESH_LT,
                    )

                    # ---- spike extraction on GPSIMD + output DMA ----
                    pieces = (
                        [(0, ch // 2), (ch // 2, ch - ch // 2)]
                        if ic == len(CH_LIST) - 1 and ch >= 8
                        else [(0, ch)]
                    )
                    for c0, clen in pieces:
                        for lane in range(LANES):
                            nc.gpsimd.tensor_scalar(
                                St[pc][:, lane, c0 : c0 + clen],
                                U[pc][:, c0 + 1 : c0 + 1 + clen, lane],
                                1.0,
                                None,
                                mybir.AluOpType.is_gt,
                            )
                        for b in range(B_SH):
                            nc.sync.dma_start(
                                out_r[b][:, :, t0 + c0 : t0 + c0 + clen],
                                St[pc][:, b * N_OT : (b + 1) * N_OT, c0 : c0 + clen],
                            )
                    prev_ch = ch

            if n_iter:
                with tc.For_i(0, n_iter) as _:
                    body()
            else:
                body()

    _dedupe_ldweights(nc)
    nc.compile()
    return nc


def _dedupe_ldweights(nc):
    """Remove back-to-back redundant Ldweights (the batch-inner GEMM loop
    issues 4 matmuls per weight tile; 3 of 4 emitted weight loads re-load
    identical bits). Safe when no sync info is attached and only Matmult
    instructions sit between."""

    def _key(inst):
        a = inst.ins[0]
        try:
            return (a.memory_location().name, a.offset, str(a.ap))
        except Exception:
            return None

    removed = 0
    for blk in nc.m.functions[0].blocks:
        prev_key = None
        keep = []
        for inst in blk.instructions:
            if inst.opcode == "Ldweights":
                k = _key(inst)
                plain = not inst.sync_info and k is not None
                if plain and k == prev_key:
                    removed += 1
                    continue
                prev_key = k if plain else None
            elif inst.opcode != "Matmult":
                prev_key = None
            keep.append(inst)
        blk.instructions = keep
    return removed


_NC_CACHE = None


def _prep_inputs(input_spikes_seq: np.ndarray, W: np.ndarray):
    W32 = np.ascontiguousarray(np.asarray(W, dtype=np.float32).T)   # [n_in, n_out]
    w_hi = W32.astype(ml_dtypes.bfloat16)
    r1 = W32 - w_hi.astype(np.float32)
    w_mid = r1.astype(ml_dtypes.bfloat16)
    wts = np.ascontiguousarray(np.stack([w_hi, w_mid]))

    x = np.asarray(input_spikes_seq, dtype=np.float32)
    in_maps = []
    for c in range(N_CORES):
        xs = x[c * B_SH : (c + 1) * B_SH]                           # [4, T, n_in]
        xs_T = np.ascontiguousarray(xs.transpose(0, 2, 1)).astype(ml_dtypes.bfloat16)
        in_maps.append({"xT": xs_T, "wts": wts})
    return in_maps


def kernel(input_spikes_seq: np.ndarray, W: np.ndarray) -> np.ndarray:
    global _NC_CACHE
    if _NC_CACHE is None:
        _NC_CACHE = _build_kernel()
    nc = _NC_CACHE

    in_maps = _prep_inputs(input_spikes_seq, W)
    res = run_bass_kernel_spmd(nc, in_maps, core_ids=list(range(N_CORES)))

    # ---- gather/unshard: [core][4, n_out, T] -> (B, T, n_out) ----
    outs = [r["out"] for r in res.results]
    full = np.concatenate(outs, axis=0)                             # [B, n_out, T]
    return np.ascontiguousarray(full.transpose(0, 2, 1))
